# revision 14
# baseline (speedup 1.0000x reference)
"""Trainium2 Bass kernel for nn_MemoryModule (vq_codebook).

reference semantics (N=16384, D=128, P=256):
    s         = repres @ memory.T                      [N, P]
    attention = softmax(s, axis=1)
    output    = attention @ memory                     [N, D]
    t1, t2    = top-2 indices of attention (== top-2 of s; softmax monotone)
    d_i       = ||repres - memory[t_i]|| / D
    loss      = mean(d1) + masked-mean(d1-d2+1e-3 | <0) + ||memory||_F
    out       = concat([output, repres], axis=1)       [N, 2D]
    mask      = t1[:, None] == t1[None, :]             [N, N] bool

Sharding: data-parallel over rows, 2048 rows/core on 8 cores.

Two SPMD launches:
  A (stats):  per 128-row tile: one matmul for scores (row-major) feeding
     max/max_index (top-2 values+indices), two matmuls for transposed scores
     feeding exp() (softmax numerator already in [P, rows] layout so it can be
     the stationary operand of the readout matmul), readout matmul against
     memory augmented with a ones column (gives the softmax denominator for
     free), normalize.  exp() needs no max-subtraction: |scores| <~ 25.
  host: top-1/2 values+indices -> distances via
     d^2 = ||x||^2 - 2*max(s) + ||m_t||^2, scalar loss; gather t1 of all rows.
  B (mask):  replicate t1(all rows) across partitions once, then one
     tensor_scalar is_equal per 128-row tile -> [128, N] uint8, DMA out.
     This is HBM-write-bound: 33.5 MB/core.
"""

import functools
import sys
from contextlib import ExitStack

import numpy as np

sys.path.insert(0, "/opt/trn_rl_repo")

import concourse.bass as bass
import concourse.bacc as bacc
import concourse.mybir as mybir
from concourse import tile
from concourse.bass_utils import run_bass_kernel_spmd

N, D, P = 16384, 128, 256
NCORES = 8
ROWS = N // NCORES      # 2048 rows per core
RT = ROWS // 128        # 16 tiles of 128 rows per core

f32 = mybir.dt.float32
u8 = mybir.dt.uint8
u32 = mybir.dt.uint32
AF = mybir.ActivationFunctionType
ALU = mybir.AluOpType

# How many of each 8 consecutive mask tiles run on DVE (rest on GPSIMD).
DVE_PER_8 = 5

TRACE = False
LAST = {}
LAST_RESULTS = {}


def enable_profiling():
    """Register the NTFF profile hook (needed for trace=True under axon)."""
    global TRACE
    try:
        import importlib.util
        import types

        if "antenv.axon_hooks" not in sys.modules:
            mod = types.ModuleType("antenv.axon_hooks")
            mod._HOOK = None
            mod.set_axon_ntff_profile_hook = lambda h: setattr(mod, "_HOOK", h)
            mod.get_axon_ntff_profile_hook = lambda: mod._HOOK
            sys.modules["antenv.axon_hooks"] = mod
            import antenv

            antenv.axon_hooks = mod

        spec = importlib.util.spec_from_file_location(
            "trn_boot", "/root/.axon_site/trn_agent_boot/trn_boot.py"
        )
        trn_boot = importlib.util.module_from_spec(spec)
        spec.loader.exec_module(trn_boot)

        hook = trn_boot._ntff_profile_via_ctypes("/opt/axon/libaxon_pjrt.so")
        sys.modules["antenv.axon_hooks"].set_axon_ntff_profile_hook(hook)
        TRACE = True
        return True
    except Exception as e:  # profiling is best-effort
        print(f"enable_profiling failed: {e}")
        return False


@functools.lru_cache(maxsize=None)
def _stats_prog():
    nc = bacc.Bacc("TRN2")
    xT = nc.declare_dram_parameter("xT", [D, ROWS], f32, isOutput=False)
    memT = nc.declare_dram_parameter("memT", [D, P], f32, isOutput=False)
    # maug[p, h, :] = concat(memory, ones)[h*128 + p, :]
    maug = nc.declare_dram_parameter("maug", [128, 2, D + 1], f32, isOutput=False)
    # raw readout + denominator column; row t*128+p lives at [p, t, :]
    out1r = nc.declare_dram_parameter("out1r", [128, RT, D + 1], f32, isOutput=True)
    vals = nc.declare_dram_parameter("vals", [128, RT * 2], f32, isOutput=True)
    idxs = nc.declare_dram_parameter("idxs", [128, RT * 2], u32, isOutput=True)

    with ExitStack() as ctx:
        tc = ctx.enter_context(tile.TileContext(nc))
        const = ctx.enter_context(tc.tile_pool(name="const", bufs=1))
        sp = ctx.enter_context(tc.tile_pool(name="sp", bufs=3))
        pp = ctx.enter_context(tc.tile_pool(name="pp", bufs=3))
        st = ctx.enter_context(tc.tile_pool(name="st", bufs=1))
        psw = ctx.enter_context(tc.tile_pool(name="psw", bufs=1, space="PSUM"))
        ps = ctx.enter_context(tc.tile_pool(name="ps", bufs=3, space="PSUM"))
        ps2 = ctx.enter_context(tc.tile_pool(name="ps2", bufs=2, space="PSUM"))

        memT_sb = const.tile([D, P], f32)
        nc.sync.dma_start(memT_sb[:], memT[:])
        maug_sb = const.tile([128, 2, D + 1], f32)
        nc.sync.dma_start(maug_sb[:], maug[:])
        xT_sb = const.tile([D, ROWS], f32)
        nc.sync.dma_start(xT_sb[:], xT[:])

        # PE warmup: observe each input DMA lane once, so no real matmul
        # ever needs more than one sync wait (HW limit: 1 per matmul).
        warm = psw.tile([1, 1], f32, tag="warm")
        nc.tensor.matmul(warm[:], memT_sb[0:1, 0:1], memT_sb[0:1, 0:1],
                         start=True, stop=True)
        nc.tensor.matmul(warm[:], maug_sb[0:1, 0, 0:1], maug_sb[0:1, 0, 0:1],
                         start=True, stop=True)
        nc.tensor.matmul(warm[:], xT_sb[0:1, 0:1], xT_sb[0:1, 0:1],
                         start=True, stop=True)

        vals_sb = st.tile([128, RT * 2], f32)
        idxs_sb = st.tile([128, RT * 2], u32)
        o_blk = st.tile([128, RT, D + 1], f32)

        for t in range(RT):
            xs = xT_sb[:, t * 128:(t + 1) * 128]

            # scores, row-major: [128 rows, 256 f]
            ps_s = ps.tile([128, P], f32, tag="ps_s")
            nc.tensor.matmul(ps_s[:], xs, memT_sb[:], start=True, stop=True)
            # all PSUM readers stay on ACT so PE's ACT wait covers slot reuse
            s_sb = sp.tile([128, P], f32)
            nc.scalar.copy(s_sb[:], ps_s[:])

            top8 = sp.tile([128, 8], f32)
            nc.vector.max(top8[:], s_sb[:])
            idx8 = sp.tile([128, 8], u32)
            nc.vector.max_index(idx8[:], top8[:], s_sb[:])
            nc.vector.tensor_copy(vals_sb[:, t * 2:t * 2 + 2], top8[:, 0:2])
            nc.vector.tensor_copy(idxs_sb[:, t * 2:t * 2 + 2], idx8[:, 0:2])

            # scores, transposed: [256 f, 128 rows] in one PSUM bank -> exp
            ps_sT = ps2.tile([128, 2, 128], f32, tag="ps_sT")
            for h in range(2):
                nc.tensor.matmul(
                    ps_sT[:, h, :], memT_sb[:, h * 128:(h + 1) * 128], xs,
                    start=True, stop=True,
                )
            pT = pp.tile([128, 2, 128], f32)
            nc.scalar.activation(pT[:], ps_sT[:], AF.Exp)

            # readout + denominator: [128 rows, 129]
            ps_o = ps2.tile([128, D + 1], f32, tag="ps_o")
            nc.tensor.matmul(ps_o[:], pT[:, 0, :], maug_sb[:, 0, :], start=True, stop=False)
            nc.tensor.matmul(ps_o[:], pT[:, 1, :], maug_sb[:, 1, :], start=False, stop=True)
            nc.scalar.copy(o_blk[:, t, :], ps_o[:])

        nc.sync.dma_start(out1r[:], o_blk[:])
        nc.sync.dma_start(vals[:], vals_sb[:])
        nc.sync.dma_start(idxs[:], idxs_sb[:])
    nc.compile()
    return nc


@functools.lru_cache(maxsize=None)
def _mask_prog():
    nc = bacc.Bacc("TRN2")
    t1all = nc.declare_dram_parameter("t1all", [1, N], u8, isOutput=False)
    # t1own[p, t] = t1 of row t*128+p of this core's block
    t1own = nc.declare_dram_parameter("t1own", [128, RT], f32, isOutput=False)
    mask = nc.declare_dram_parameter("mask", [ROWS, N], u8, isOutput=True)

    with ExitStack() as ctx:
        tc = ctx.enter_context(tile.TileContext(nc))
        const = ctx.enter_context(tc.tile_pool(name="const", bufs=1))
        mp = ctx.enter_context(tc.tile_pool(name="mp", bufs=4))

        t1rep = const.tile([128, N], u8)
        nc.sync.dma_start(t1rep[:], t1all[0:1, :].partition_broadcast(128))
        t1own_sb = const.tile([128, RT], f32)
        nc.sync.dma_start(t1own_sb[:], t1own[:])

        # warmups: let each compute engine observe the input DMA lanes once
        wa = const.tile([128, 8], u8)
        nc.vector.tensor_copy(wa[:], t1rep[:, 0:8])
        wb = const.tile([128, 1], f32)
        nc.vector.tensor_copy(wb[:], t1own_sb[:, 0:1])
        wc = const.tile([128, 8], u8)
        nc.gpsimd.tensor_copy(wc[:], t1rep[:, 0:8])
        wd = const.tile([128, 1], f32)
        nc.gpsimd.tensor_copy(wd[:], t1own_sb[:, 0:1])

        for t in range(RT):
            m_sb = mp.tile([128, N], u8)
            eng = nc.vector if (t % 8) < DVE_PER_8 else nc.gpsimd
            eng.tensor_scalar(
                m_sb[:], t1rep[:], t1own_sb[:, t:t + 1], None, ALU.is_equal
            )
            nc.sync.dma_start(mask[t * 128:(t + 1) * 128, :], m_sb[:])
    nc.compile()
    return nc


def _run(nc, in_maps, label):
    res = run_bass_kernel_spmd(nc, in_maps, list(range(NCORES)), trace=TRACE)
    if TRACE:
        LAST[label] = res.exec_time_ns
        LAST_RESULTS[label] = res
    return res.results


def kernel(repres, memory):
    repres = np.ascontiguousarray(np.asarray(repres, dtype=np.float32))
    memory = np.ascontiguousarray(np.asarray(memory, dtype=np.float32))

    memT = np.ascontiguousarray(memory.T)                                  # [128, 256]
    maug = np.concatenate([memory, np.ones((P, 1), np.float32)], axis=1)   # [256, 129]
    maug = np.ascontiguousarray(maug.reshape(2, 128, D + 1).transpose(1, 0, 2))

    in_maps = [
        {
            "xT": np.ascontiguousarray(repres[c * ROWS:(c + 1) * ROWS].T),
            "memT": memT,
            "maug": maug,
        }
        for c in range(NCORES)
    ]
    resA = _run(_stats_prog(), in_maps, "stats")

    out1 = np.empty((N, D), np.float32)
    m12 = np.empty((N, 2), np.float32)
    t12 = np.empty((N, 2), np.int64)
    for c in range(NCORES):
        r = resA[c]
        sl = slice(c * ROWS, (c + 1) * ROWS)
        raw = r["out1r"].transpose(1, 0, 2).reshape(ROWS, D + 1)
        out1[sl] = raw[:, :D] / raw[:, D:D + 1]
        m12[sl] = r["vals"].reshape(128, RT, 2).transpose(1, 0, 2).reshape(ROWS, 2)
        t12[sl] = (
            r["idxs"].reshape(128, RT, 2).transpose(1, 0, 2).reshape(ROWS, 2)
        )

    t1 = t12[:, 0]
    t2 = t12[:, 1]

    # distances from score stats: d^2 = ||x||^2 - 2*s[t] + ||m_t||^2
    r2 = np.einsum("nd,nd->n", repres, repres).astype(np.float32)
    mn2 = np.einsum("pd,pd->p", memory, memory).astype(np.float32)
    d1 = np.sqrt(np.maximum(r2 - 2.0 * m12[:, 0] + mn2[t1], 0.0).astype(np.float32))
    d2 = np.sqrt(np.maximum(r2 - 2.0 * m12[:, 1] + mn2[t2], 0.0).astype(np.float32))
    d1 = (d1 / np.float32(D)).astype(np.float32)
    d2 = (d2 / np.float32(D)).astype(np.float32)

    loss = np.float32(np.mean(d1))
    diff = (d1 - d2 + np.float32(0.001)).astype(np.float32)
    neg = diff < 0
    cnt = int(neg.sum())
    if cnt > 0:
        loss = np.float32(loss + np.float32(diff[neg].sum()) / np.float32(cnt))
    loss = np.float32(loss + np.float32(np.sqrt(np.sum(memory * memory))))

    out = np.concatenate([out1, repres], axis=1)

    t1u8 = np.ascontiguousarray(t1.astype(np.uint8).reshape(1, N))
    in_maps = [
        {
            "t1all": t1u8,
            "t1own": np.ascontiguousarray(
                t1u8.reshape(N)[c * ROWS:(c + 1) * ROWS]
                .reshape(RT, 128).T.astype(np.float32)
            ),
        }
        for c in range(NCORES)
    ]
    resB = _run(_mask_prog(), in_maps, "mask")

    mask = np.empty((N, N), np.uint8)
    for c in range(NCORES):
        mask[c * ROWS:(c + 1) * ROWS] = resB[c]["mask"]
    return out, loss, mask.view(np.bool_)


# revision 15
# speedup vs baseline: 6.5051x; 6.5051x over previous
"""Trainium2 Bass kernel for nn_MemoryModule (vq_codebook).

reference semantics (N=16384, D=128, P=256):
    s         = repres @ memory.T                      [N, P]
    attention = softmax(s, axis=1)
    output    = attention @ memory                     [N, D]
    t1, t2    = top-2 indices of attention (== top-2 of s; softmax monotone)
    d_i       = ||repres - memory[t_i]|| / D
    loss      = mean(d1) + masked-mean(d1-d2+1e-3 | <0) + ||memory||_F
    out       = concat([output, repres], axis=1)       [N, 2D]
    mask      = t1[:, None] == t1[None, :]             [N, N] bool

Sharding: data-parallel over rows, 2048 rows/core on 8 cores.

Two SPMD launches:
  A (stats):  per 128-row tile: one matmul for scores (row-major) feeding
     max/max_index (top-2 values+indices), two matmuls for transposed scores
     feeding exp() (softmax numerator already in [P, rows] layout so it can be
     the stationary operand of the readout matmul), readout matmul against
     memory augmented with a ones column (gives the softmax denominator for
     free), normalize.  exp() needs no max-subtraction: |scores| <~ 25.
  host: top-1/2 values+indices -> distances via
     d^2 = ||x||^2 - 2*max(s) + ||m_t||^2, scalar loss; gather t1 of all rows.
  B (mask):  replicate t1(all rows) across partitions once, then one
     tensor_scalar is_equal per 128-row tile -> [128, N] uint8, DMA out.
     This is HBM-write-bound: 33.5 MB/core.
"""

import functools
import sys
from contextlib import ExitStack

import numpy as np

sys.path.insert(0, "/opt/trn_rl_repo")

import concourse.bass as bass
import concourse.bacc as bacc
import concourse.mybir as mybir
from concourse import tile
from concourse.bass_utils import run_bass_kernel_spmd

N, D, P = 16384, 128, 256
NCORES = 8
ROWS = N // NCORES      # 2048 rows per core
RT = ROWS // 128        # 16 tiles of 128 rows per core

f32 = mybir.dt.float32
bf16 = mybir.dt.bfloat16
u8 = mybir.dt.uint8
u32 = mybir.dt.uint32
AF = mybir.ActivationFunctionType
ALU = mybir.AluOpType

TRACE = False
LAST = {}
LAST_RESULTS = {}


def enable_profiling():
    """Register the NTFF profile hook (needed for trace=True under axon)."""
    global TRACE
    try:
        import importlib.util
        import types

        if "antenv.axon_hooks" not in sys.modules:
            mod = types.ModuleType("antenv.axon_hooks")
            mod._HOOK = None
            mod.set_axon_ntff_profile_hook = lambda h: setattr(mod, "_HOOK", h)
            mod.get_axon_ntff_profile_hook = lambda: mod._HOOK
            sys.modules["antenv.axon_hooks"] = mod
            import antenv

            antenv.axon_hooks = mod

        spec = importlib.util.spec_from_file_location(
            "trn_boot", "/root/.axon_site/trn_agent_boot/trn_boot.py"
        )
        trn_boot = importlib.util.module_from_spec(spec)
        spec.loader.exec_module(trn_boot)

        hook = trn_boot._ntff_profile_via_ctypes("/opt/axon/libaxon_pjrt.so")
        sys.modules["antenv.axon_hooks"].set_axon_ntff_profile_hook(hook)
        TRACE = True
        return True
    except Exception as e:  # profiling is best-effort
        print(f"enable_profiling failed: {e}")
        return False


@functools.lru_cache(maxsize=None)
def _stats_prog():
    nc = bacc.Bacc("TRN2")
    xT = nc.declare_dram_parameter("xT", [D, ROWS], f32, isOutput=False)
    memT = nc.declare_dram_parameter("memT", [D, P], f32, isOutput=False)
    # maug[p, h, :] = concat(memory, ones)[h*128 + p, :]
    maug = nc.declare_dram_parameter("maug", [128, 2, D + 1], f32, isOutput=False)
    # raw readout + denominator column; row t*128+p lives at [p, t, :]
    out1r = nc.declare_dram_parameter("out1r", [128, RT, D + 1], f32, isOutput=True)
    vals = nc.declare_dram_parameter("vals", [128, RT * 2], f32, isOutput=True)
    idxs = nc.declare_dram_parameter("idxs", [128, RT * 2], u32, isOutput=True)

    with ExitStack() as ctx:
        tc = ctx.enter_context(tile.TileContext(nc))
        const = ctx.enter_context(tc.tile_pool(name="const", bufs=1))
        sp = ctx.enter_context(tc.tile_pool(name="sp", bufs=3))
        pp = ctx.enter_context(tc.tile_pool(name="pp", bufs=3))
        st = ctx.enter_context(tc.tile_pool(name="st", bufs=1))
        psw = ctx.enter_context(tc.tile_pool(name="psw", bufs=1, space="PSUM"))
        ps = ctx.enter_context(tc.tile_pool(name="ps", bufs=3, space="PSUM"))
        ps2 = ctx.enter_context(tc.tile_pool(name="ps2", bufs=2, space="PSUM"))

        memT_sb = const.tile([D, P], f32)
        nc.sync.dma_start(memT_sb[:], memT[:])
        maug_sb = const.tile([128, 2, D + 1], f32)
        nc.sync.dma_start(maug_sb[:], maug[:])
        xT_sb = const.tile([D, ROWS], f32)
        nc.sync.dma_start(xT_sb[:], xT[:])

        # PE warmup: observe each input DMA lane once, so no real matmul
        # ever needs more than one sync wait (HW limit: 1 per matmul).
        warm = psw.tile([1, 1], f32, tag="warm")
        nc.tensor.matmul(warm[:], memT_sb[0:1, 0:1], memT_sb[0:1, 0:1],
                         start=True, stop=True)
        nc.tensor.matmul(warm[:], maug_sb[0:1, 0, 0:1], maug_sb[0:1, 0, 0:1],
                         start=True, stop=True)
        nc.tensor.matmul(warm[:], xT_sb[0:1, 0:1], xT_sb[0:1, 0:1],
                         start=True, stop=True)

        vals_sb = st.tile([128, RT * 2], f32)
        idxs_sb = st.tile([128, RT * 2], u32)
        o_blk = st.tile([128, RT, D + 1], f32)

        for t in range(RT):
            xs = xT_sb[:, t * 128:(t + 1) * 128]

            # scores, row-major: [128 rows, 256 f]
            ps_s = ps.tile([128, P], f32, tag="ps_s")
            nc.tensor.matmul(ps_s[:], xs, memT_sb[:], start=True, stop=True)
            # all PSUM readers stay on ACT so PE's ACT wait covers slot reuse
            s_sb = sp.tile([128, P], f32)
            nc.scalar.copy(s_sb[:], ps_s[:])

            top8 = sp.tile([128, 8], f32)
            nc.vector.max(top8[:], s_sb[:])
            idx8 = sp.tile([128, 8], u32)
            nc.vector.max_index(idx8[:], top8[:], s_sb[:])
            nc.vector.tensor_copy(vals_sb[:, t * 2:t * 2 + 2], top8[:, 0:2])
            nc.vector.tensor_copy(idxs_sb[:, t * 2:t * 2 + 2], idx8[:, 0:2])

            # scores, transposed: [256 f, 128 rows] in one PSUM bank -> exp
            ps_sT = ps2.tile([128, 2, 128], f32, tag="ps_sT")
            for h in range(2):
                nc.tensor.matmul(
                    ps_sT[:, h, :], memT_sb[:, h * 128:(h + 1) * 128], xs,
                    start=True, stop=True,
                )
            pT = pp.tile([128, 2, 128], f32)
            nc.scalar.activation(pT[:], ps_sT[:], AF.Exp)

            # readout + denominator: [128 rows, 129]
            ps_o = ps2.tile([128, D + 1], f32, tag="ps_o")
            nc.tensor.matmul(ps_o[:], pT[:, 0, :], maug_sb[:, 0, :], start=True, stop=False)
            nc.tensor.matmul(ps_o[:], pT[:, 1, :], maug_sb[:, 1, :], start=False, stop=True)
            nc.scalar.copy(o_blk[:, t, :], ps_o[:])

        nc.sync.dma_start(out1r[:], o_blk[:])
        nc.sync.dma_start(vals[:], vals_sb[:])
        nc.sync.dma_start(idxs[:], idxs_sb[:])
    nc.compile()
    return nc


# tiles handled by DVE is_equal; the rest go to ACT via the exact integer
# indicator relu(1 - (x-c)^2) (two activations). DVE ~11.7us/tile,
# ACT ~2x13.9us/tile.
DVE_TILES = 11


@functools.lru_cache(maxsize=None)
def _mask_prog():
    nc = bacc.Bacc("TRN2")
    t1all = nc.declare_dram_parameter("t1all", [1, N], u8, isOutput=False)
    # t1own[p, t, 0] = t1 of row t*128+p of this core's block; [.., 1] = -t1
    t1own = nc.declare_dram_parameter("t1own", [128, RT, 2], f32, isOutput=False)
    mask = nc.declare_dram_parameter("mask", [ROWS, N], u8, isOutput=True)

    with ExitStack() as ctx:
        tc = ctx.enter_context(tile.TileContext(nc))
        const = ctx.enter_context(tc.tile_pool(name="const", bufs=1))
        mpd = ctx.enter_context(tc.tile_pool(name="mpd", bufs=3))
        mpa = ctx.enter_context(tc.tile_pool(name="mpa", bufs=3))
        sqp = ctx.enter_context(tc.tile_pool(name="sqp", bufs=2))

        t1rep = const.tile([128, N], u8)
        nc.sync.dma_start(t1rep[:], t1all[0:1, :].partition_broadcast(128))
        t1own_sb = const.tile([128, RT, 2], f32)
        nc.sync.dma_start(t1own_sb[:], t1own[:])

        # warmups: let each compute engine observe the input DMA lanes once
        wa = const.tile([128, 8], u8)
        nc.vector.tensor_copy(wa[:], t1rep[:, 0:8])
        wb = const.tile([128, 1], f32)
        nc.vector.tensor_copy(wb[:], t1own_sb[:, 0:1, 0])
        wc = const.tile([128, 8], bf16)
        nc.scalar.activation(wc[:], t1rep[:, 0:8], AF.Square,
                             bias=t1own_sb[:, 0:1, 1])
        wd = const.tile([128, 1], f32)
        nc.scalar.copy(wd[:], t1own_sb[:, 0:1, 0])

        for t in range(RT):
            if t < DVE_TILES:
                m_sb = mpd.tile([128, N], u8, tag="md")
                nc.vector.tensor_scalar(
                    m_sb[:], t1rep[:], t1own_sb[:, t:t + 1, 0], None, ALU.is_equal
                )
            else:
                sq = sqp.tile([128, N], bf16, tag="sq")
                nc.scalar.activation(sq[:], t1rep[:], AF.Square,
                                     bias=t1own_sb[:, t:t + 1, 1])
                m_sb = mpa.tile([128, N], u8, tag="ma")
                nc.scalar.activation(m_sb[:], sq[:], AF.Relu,
                                     bias=1.0, scale=-1.0)
            nc.sync.dma_start(mask[t * 128:(t + 1) * 128, :], m_sb[:])
    nc.compile()
    return nc


def _run(nc, in_maps, label):
    res = run_bass_kernel_spmd(nc, in_maps, list(range(NCORES)), trace=TRACE)
    if TRACE:
        LAST[label] = res.exec_time_ns
        LAST_RESULTS[label] = res
    return res.results


def kernel(repres, memory):
    repres = np.ascontiguousarray(np.asarray(repres, dtype=np.float32))
    memory = np.ascontiguousarray(np.asarray(memory, dtype=np.float32))

    memT = np.ascontiguousarray(memory.T)                                  # [128, 256]
    maug = np.concatenate([memory, np.ones((P, 1), np.float32)], axis=1)   # [256, 129]
    maug = np.ascontiguousarray(maug.reshape(2, 128, D + 1).transpose(1, 0, 2))

    in_maps = [
        {
            "xT": np.ascontiguousarray(repres[c * ROWS:(c + 1) * ROWS].T),
            "memT": memT,
            "maug": maug,
        }
        for c in range(NCORES)
    ]
    resA = _run(_stats_prog(), in_maps, "stats")

    out1 = np.empty((N, D), np.float32)
    m12 = np.empty((N, 2), np.float32)
    t12 = np.empty((N, 2), np.int64)
    for c in range(NCORES):
        r = resA[c]
        sl = slice(c * ROWS, (c + 1) * ROWS)
        raw = r["out1r"].transpose(1, 0, 2).reshape(ROWS, D + 1)
        out1[sl] = raw[:, :D] / raw[:, D:D + 1]
        m12[sl] = r["vals"].reshape(128, RT, 2).transpose(1, 0, 2).reshape(ROWS, 2)
        t12[sl] = (
            r["idxs"].reshape(128, RT, 2).transpose(1, 0, 2).reshape(ROWS, 2)
        )

    t1 = t12[:, 0]
    t2 = t12[:, 1]

    # distances from score stats: d^2 = ||x||^2 - 2*s[t] + ||m_t||^2
    r2 = np.einsum("nd,nd->n", repres, repres).astype(np.float32)
    mn2 = np.einsum("pd,pd->p", memory, memory).astype(np.float32)
    d1 = np.sqrt(np.maximum(r2 - 2.0 * m12[:, 0] + mn2[t1], 0.0).astype(np.float32))
    d2 = np.sqrt(np.maximum(r2 - 2.0 * m12[:, 1] + mn2[t2], 0.0).astype(np.float32))
    d1 = (d1 / np.float32(D)).astype(np.float32)
    d2 = (d2 / np.float32(D)).astype(np.float32)

    loss = np.float32(np.mean(d1))
    diff = (d1 - d2 + np.float32(0.001)).astype(np.float32)
    neg = diff < 0
    cnt = int(neg.sum())
    if cnt > 0:
        loss = np.float32(loss + np.float32(diff[neg].sum()) / np.float32(cnt))
    loss = np.float32(loss + np.float32(np.sqrt(np.sum(memory * memory))))

    out = np.concatenate([out1, repres], axis=1)

    t1u8 = np.ascontiguousarray(t1.astype(np.uint8).reshape(1, N))
    own_blocks = [
        t1[c * ROWS:(c + 1) * ROWS].reshape(RT, 128).T.astype(np.float32)
        for c in range(NCORES)
    ]
    in_maps = [
        {
            "t1all": t1u8,
            "t1own": np.ascontiguousarray(
                np.stack([ownT, -ownT], axis=-1).astype(np.float32)
            ),
        }
        for ownT in own_blocks
    ]
    resB = _run(_mask_prog(), in_maps, "mask")

    mask = np.empty((N, N), np.uint8)
    for c in range(NCORES):
        mask[c * ROWS:(c + 1) * ROWS] = resB[c]["mask"]
    return out, loss, mask.view(np.bool_)


# revision 16
# speedup vs baseline: 8.9326x; 1.3732x over previous
"""Trainium2 Bass kernel for nn_MemoryModule (vq_codebook).

reference semantics (N=16384, D=128, P=256):
    s         = repres @ memory.T                      [N, P]
    attention = softmax(s, axis=1)
    output    = attention @ memory                     [N, D]
    t1, t2    = top-2 indices of attention (== top-2 of s; softmax monotone)
    d_i       = ||repres - memory[t_i]|| / D
    loss      = mean(d1) + masked-mean(d1-d2+1e-3 | <0) + ||memory||_F
    out       = concat([output, repres], axis=1)       [N, 2D]
    mask      = t1[:, None] == t1[None, :]             [N, N] bool

Sharding: data-parallel over rows, 2048 rows/core on 8 cores.

Two SPMD launches:
  A (stats):  per 128-row tile: one matmul for scores (row-major) feeding
     max/max_index (top-2 values+indices), two matmuls for transposed scores
     feeding exp() (softmax numerator already in [P, rows] layout so it can be
     the stationary operand of the readout matmul), readout matmul against
     memory augmented with a ones column (gives the softmax denominator for
     free), normalize.  exp() needs no max-subtraction: |scores| <~ 25.
  host: top-1/2 values+indices -> distances via
     d^2 = ||x||^2 - 2*max(s) + ||m_t||^2, scalar loss; gather t1 of all rows.
  B (mask):  replicate t1(all rows) across partitions once, then one
     tensor_scalar is_equal per 128-row tile -> [128, N] uint8, DMA out.
     This is HBM-write-bound: 33.5 MB/core.
"""

import functools
import sys
from contextlib import ExitStack

import numpy as np

sys.path.insert(0, "/opt/trn_rl_repo")

import concourse.bass as bass
import concourse.bacc as bacc
import concourse.mybir as mybir
from concourse import tile
from concourse.bass_utils import run_bass_kernel_spmd

N, D, P = 16384, 128, 256
NCORES = 8
ROWS = N // NCORES      # 2048 rows per core
RT = ROWS // 128        # 16 tiles of 128 rows per core

f32 = mybir.dt.float32
bf16 = mybir.dt.bfloat16
u8 = mybir.dt.uint8
u32 = mybir.dt.uint32
AF = mybir.ActivationFunctionType
ALU = mybir.AluOpType

TRACE = False
LAST = {}
LAST_RESULTS = {}


def enable_profiling():
    """Register the NTFF profile hook (needed for trace=True under axon)."""
    global TRACE
    try:
        import importlib.util
        import types

        if "antenv.axon_hooks" not in sys.modules:
            mod = types.ModuleType("antenv.axon_hooks")
            mod._HOOK = None
            mod.set_axon_ntff_profile_hook = lambda h: setattr(mod, "_HOOK", h)
            mod.get_axon_ntff_profile_hook = lambda: mod._HOOK
            sys.modules["antenv.axon_hooks"] = mod
            import antenv

            antenv.axon_hooks = mod

        spec = importlib.util.spec_from_file_location(
            "trn_boot", "/root/.axon_site/trn_agent_boot/trn_boot.py"
        )
        trn_boot = importlib.util.module_from_spec(spec)
        spec.loader.exec_module(trn_boot)

        hook = trn_boot._ntff_profile_via_ctypes("/opt/axon/libaxon_pjrt.so")
        sys.modules["antenv.axon_hooks"].set_axon_ntff_profile_hook(hook)
        TRACE = True
        return True
    except Exception as e:  # profiling is best-effort
        print(f"enable_profiling failed: {e}")
        return False


@functools.lru_cache(maxsize=None)
def _stats_prog():
    nc = bacc.Bacc("TRN2")
    xT = nc.declare_dram_parameter("xT", [D, ROWS], f32, isOutput=False)
    memT = nc.declare_dram_parameter("memT", [D, P], f32, isOutput=False)
    # maug[p, h, :] = concat(memory, ones)[h*128 + p, :]
    maug = nc.declare_dram_parameter("maug", [128, 2, D + 1], f32, isOutput=False)
    # raw readout + denominator column; row t*128+p lives at [p, t, :]
    out1r = nc.declare_dram_parameter("out1r", [128, RT, D + 1], f32, isOutput=True)
    vals = nc.declare_dram_parameter("vals", [128, RT * 2], f32, isOutput=True)
    idxs = nc.declare_dram_parameter("idxs", [128, RT * 2], u32, isOutput=True)

    with ExitStack() as ctx:
        tc = ctx.enter_context(tile.TileContext(nc))
        const = ctx.enter_context(tc.tile_pool(name="const", bufs=1))
        sp = ctx.enter_context(tc.tile_pool(name="sp", bufs=3))
        pp = ctx.enter_context(tc.tile_pool(name="pp", bufs=3))
        st = ctx.enter_context(tc.tile_pool(name="st", bufs=1))
        psw = ctx.enter_context(tc.tile_pool(name="psw", bufs=1, space="PSUM"))
        ps = ctx.enter_context(tc.tile_pool(name="ps", bufs=3, space="PSUM"))
        ps2 = ctx.enter_context(tc.tile_pool(name="ps2", bufs=2, space="PSUM"))

        memT_sb = const.tile([D, P], f32)
        nc.sync.dma_start(memT_sb[:], memT[:])
        maug_sb = const.tile([128, 2, D + 1], f32)
        nc.sync.dma_start(maug_sb[:], maug[:])
        xT_sb = const.tile([D, ROWS], f32)
        nc.sync.dma_start(xT_sb[:], xT[:])

        # PE warmup: observe each input DMA lane once, so no real matmul
        # ever needs more than one sync wait (HW limit: 1 per matmul).
        warm = psw.tile([1, 1], f32, tag="warm")
        nc.tensor.matmul(warm[:], memT_sb[0:1, 0:1], memT_sb[0:1, 0:1],
                         start=True, stop=True)
        nc.tensor.matmul(warm[:], maug_sb[0:1, 0, 0:1], maug_sb[0:1, 0, 0:1],
                         start=True, stop=True)
        nc.tensor.matmul(warm[:], xT_sb[0:1, 0:1], xT_sb[0:1, 0:1],
                         start=True, stop=True)

        vals_sb = st.tile([128, RT * 2], f32)
        idxs_sb = st.tile([128, RT * 2], u32)
        o_blk = st.tile([128, RT, D + 1], f32)

        for t in range(RT):
            xs = xT_sb[:, t * 128:(t + 1) * 128]

            # scores, row-major: [128 rows, 256 f]
            ps_s = ps.tile([128, P], f32, tag="ps_s")
            nc.tensor.matmul(ps_s[:], xs, memT_sb[:], start=True, stop=True)
            # all PSUM readers stay on ACT so PE's ACT wait covers slot reuse
            s_sb = sp.tile([128, P], f32)
            nc.scalar.copy(s_sb[:], ps_s[:])

            top8 = sp.tile([128, 8], f32)
            nc.vector.max(top8[:], s_sb[:])
            idx8 = sp.tile([128, 8], u32)
            nc.vector.max_index(idx8[:], top8[:], s_sb[:])
            nc.vector.tensor_copy(vals_sb[:, t * 2:t * 2 + 2], top8[:, 0:2])
            nc.vector.tensor_copy(idxs_sb[:, t * 2:t * 2 + 2], idx8[:, 0:2])

            # scores, transposed: [256 f, 128 rows] in one PSUM bank -> exp
            ps_sT = ps2.tile([128, 2, 128], f32, tag="ps_sT")
            for h in range(2):
                nc.tensor.matmul(
                    ps_sT[:, h, :], memT_sb[:, h * 128:(h + 1) * 128], xs,
                    start=True, stop=True,
                )
            pT = pp.tile([128, 2, 128], f32)
            nc.scalar.activation(pT[:], ps_sT[:], AF.Exp)

            # readout + denominator: [128 rows, 129]
            ps_o = ps2.tile([128, D + 1], f32, tag="ps_o")
            nc.tensor.matmul(ps_o[:], pT[:, 0, :], maug_sb[:, 0, :], start=True, stop=False)
            nc.tensor.matmul(ps_o[:], pT[:, 1, :], maug_sb[:, 1, :], start=False, stop=True)
            nc.scalar.copy(o_blk[:, t, :], ps_o[:])

        nc.sync.dma_start(out1r[:], o_blk[:])
        nc.sync.dma_start(vals[:], vals_sb[:])
        nc.sync.dma_start(idxs[:], idxs_sb[:])
    nc.compile()
    return nc


# tiles handled by DVE is_equal; the rest go to ACT via the exact integer
# indicator relu(1 - (x-c)^2) (two activations). DVE ~11.7us/tile,
# ACT ~2x13.9us/tile.
DVE_TILES = 12


@functools.lru_cache(maxsize=None)
def _mask_prog():
    nc = bacc.Bacc("TRN2")
    t1all = nc.declare_dram_parameter("t1all", [1, N], u8, isOutput=False)
    # t1own[p, t, 0] = t1 of row t*128+p of this core's block; [.., 1] = -t1
    t1own = nc.declare_dram_parameter("t1own", [128, RT, 2], f32, isOutput=False)
    mask = nc.declare_dram_parameter("mask", [ROWS, N], u8, isOutput=True)

    with ExitStack() as ctx:
        tc = ctx.enter_context(tile.TileContext(nc))
        const = ctx.enter_context(tc.tile_pool(name="const", bufs=1))
        mpd = ctx.enter_context(tc.tile_pool(name="mpd", bufs=3))
        mpa = ctx.enter_context(tc.tile_pool(name="mpa", bufs=3))
        sqp = ctx.enter_context(tc.tile_pool(name="sqp", bufs=2))

        t1rep = const.tile([128, N], u8)
        nc.sync.dma_start(t1rep[:], t1all[0:1, :].partition_broadcast(128))
        t1own_sb = const.tile([128, RT, 2], f32)
        nc.sync.dma_start(t1own_sb[:], t1own[:])

        # warmups: let each compute engine observe the input DMA lanes once
        wa = const.tile([128, 8], u8)
        nc.vector.tensor_copy(wa[:], t1rep[:, 0:8])
        wb = const.tile([128, 1], f32)
        nc.vector.tensor_copy(wb[:], t1own_sb[:, 0:1, 0])
        wc = const.tile([128, 8], bf16)
        nc.scalar.activation(wc[:], t1rep[:, 0:8], AF.Square,
                             bias=t1own_sb[:, 0:1, 1])
        wd = const.tile([128, 1], f32)
        nc.scalar.copy(wd[:], t1own_sb[:, 0:1, 0])

        for t in range(RT):
            if t < DVE_TILES:
                m_sb = mpd.tile([128, N], u8, tag="md")
                nc.vector.tensor_scalar(
                    m_sb[:], t1rep[:], t1own_sb[:, t:t + 1, 0], None, ALU.is_equal
                )
            else:
                sq = sqp.tile([128, N], bf16, tag="sq")
                nc.scalar.activation(sq[:], t1rep[:], AF.Square,
                                     bias=t1own_sb[:, t:t + 1, 1])
                m_sb = mpa.tile([128, N], u8, tag="ma")
                nc.scalar.activation(m_sb[:], sq[:], AF.Relu,
                                     bias=1.0, scale=-1.0)
            nc.sync.dma_start(mask[t * 128:(t + 1) * 128, :], m_sb[:])
    nc.compile()
    return nc


def _run(nc, in_maps, label):
    res = run_bass_kernel_spmd(nc, in_maps, list(range(NCORES)), trace=TRACE)
    if TRACE:
        LAST[label] = res.exec_time_ns
        LAST_RESULTS[label] = res
    return res.results


def kernel(repres, memory):
    repres = np.ascontiguousarray(np.asarray(repres, dtype=np.float32))
    memory = np.ascontiguousarray(np.asarray(memory, dtype=np.float32))

    memT = np.ascontiguousarray(memory.T)                                  # [128, 256]
    maug = np.concatenate([memory, np.ones((P, 1), np.float32)], axis=1)   # [256, 129]
    maug = np.ascontiguousarray(maug.reshape(2, 128, D + 1).transpose(1, 0, 2))

    in_maps = [
        {
            "xT": np.ascontiguousarray(repres[c * ROWS:(c + 1) * ROWS].T),
            "memT": memT,
            "maug": maug,
        }
        for c in range(NCORES)
    ]
    resA = _run(_stats_prog(), in_maps, "stats")

    out1 = np.empty((N, D), np.float32)
    m12 = np.empty((N, 2), np.float32)
    t12 = np.empty((N, 2), np.int64)
    for c in range(NCORES):
        r = resA[c]
        sl = slice(c * ROWS, (c + 1) * ROWS)
        raw = r["out1r"].transpose(1, 0, 2).reshape(ROWS, D + 1)
        out1[sl] = raw[:, :D] / raw[:, D:D + 1]
        m12[sl] = r["vals"].reshape(128, RT, 2).transpose(1, 0, 2).reshape(ROWS, 2)
        t12[sl] = (
            r["idxs"].reshape(128, RT, 2).transpose(1, 0, 2).reshape(ROWS, 2)
        )

    t1 = t12[:, 0]
    t2 = t12[:, 1]

    # distances from score stats: d^2 = ||x||^2 - 2*s[t] + ||m_t||^2
    r2 = np.einsum("nd,nd->n", repres, repres).astype(np.float32)
    mn2 = np.einsum("pd,pd->p", memory, memory).astype(np.float32)
    d1 = np.sqrt(np.maximum(r2 - 2.0 * m12[:, 0] + mn2[t1], 0.0).astype(np.float32))
    d2 = np.sqrt(np.maximum(r2 - 2.0 * m12[:, 1] + mn2[t2], 0.0).astype(np.float32))
    d1 = (d1 / np.float32(D)).astype(np.float32)
    d2 = (d2 / np.float32(D)).astype(np.float32)

    loss = np.float32(np.mean(d1))
    diff = (d1 - d2 + np.float32(0.001)).astype(np.float32)
    neg = diff < 0
    cnt = int(neg.sum())
    if cnt > 0:
        loss = np.float32(loss + np.float32(diff[neg].sum()) / np.float32(cnt))
    loss = np.float32(loss + np.float32(np.sqrt(np.sum(memory * memory))))

    out = np.concatenate([out1, repres], axis=1)

    t1u8 = np.ascontiguousarray(t1.astype(np.uint8).reshape(1, N))
    own_blocks = [
        t1[c * ROWS:(c + 1) * ROWS].reshape(RT, 128).T.astype(np.float32)
        for c in range(NCORES)
    ]
    in_maps = [
        {
            "t1all": t1u8,
            "t1own": np.ascontiguousarray(
                np.stack([ownT, -ownT], axis=-1).astype(np.float32)
            ),
        }
        for ownT in own_blocks
    ]
    resB = _run(_mask_prog(), in_maps, "mask")

    mask = np.empty((N, N), np.uint8)
    for c in range(NCORES):
        mask[c * ROWS:(c + 1) * ROWS] = resB[c]["mask"]
    return out, loss, mask.view(np.bool_)


# revision 18
# speedup vs baseline: 8.9687x; 1.0040x over previous
"""Trainium2 Bass kernel for nn_MemoryModule (vq_codebook).

reference semantics (N=16384, D=128, P=256):
    s         = repres @ memory.T                      [N, P]
    attention = softmax(s, axis=1)
    output    = attention @ memory                     [N, D]
    t1, t2    = top-2 indices of attention (== top-2 of s; softmax monotone)
    d_i       = ||repres - memory[t_i]|| / D
    loss      = mean(d1) + masked-mean(d1-d2+1e-3 | <0) + ||memory||_F
    out       = concat([output, repres], axis=1)       [N, 2D]
    mask      = t1[:, None] == t1[None, :]             [N, N] bool

Sharding: data-parallel over rows, 2048 rows/core on 8 cores.

Two SPMD launches:
  A (stats):  per 128-row tile: one matmul for scores (row-major) feeding
     max/max_index (top-2 values+indices), two matmuls for transposed scores
     feeding exp() (softmax numerator already in [P, rows] layout so it can be
     the stationary operand of the readout matmul), readout matmul against
     memory augmented with a ones column (gives the softmax denominator for
     free), normalize.  exp() needs no max-subtraction: |scores| <~ 25.
  host: top-1/2 values+indices -> distances via
     d^2 = ||x||^2 - 2*max(s) + ||m_t||^2, scalar loss; gather t1 of all rows.
  B (mask):  replicate t1(all rows) across partitions once, then one
     tensor_scalar is_equal per 128-row tile -> [128, N] uint8, DMA out.
     This is HBM-write-bound: 33.5 MB/core.
"""

import functools
import sys
from contextlib import ExitStack

import numpy as np

sys.path.insert(0, "/opt/trn_rl_repo")

import concourse.bass as bass
import concourse.bacc as bacc
import concourse.mybir as mybir
from concourse import tile
from concourse.bass_utils import run_bass_kernel_spmd

N, D, P = 16384, 128, 256
NCORES = 8
ROWS = N // NCORES      # 2048 rows per core
RT = ROWS // 128        # 16 tiles of 128 rows per core

f32 = mybir.dt.float32
bf16 = mybir.dt.bfloat16
u8 = mybir.dt.uint8
u32 = mybir.dt.uint32
AF = mybir.ActivationFunctionType
ALU = mybir.AluOpType

TRACE = False
LAST = {}
LAST_RESULTS = {}


def enable_profiling():
    """Register the NTFF profile hook (needed for trace=True under axon)."""
    global TRACE
    try:
        import importlib.util
        import types

        if "antenv.axon_hooks" not in sys.modules:
            mod = types.ModuleType("antenv.axon_hooks")
            mod._HOOK = None
            mod.set_axon_ntff_profile_hook = lambda h: setattr(mod, "_HOOK", h)
            mod.get_axon_ntff_profile_hook = lambda: mod._HOOK
            sys.modules["antenv.axon_hooks"] = mod
            import antenv

            antenv.axon_hooks = mod

        spec = importlib.util.spec_from_file_location(
            "trn_boot", "/root/.axon_site/trn_agent_boot/trn_boot.py"
        )
        trn_boot = importlib.util.module_from_spec(spec)
        spec.loader.exec_module(trn_boot)

        hook = trn_boot._ntff_profile_via_ctypes("/opt/axon/libaxon_pjrt.so")
        sys.modules["antenv.axon_hooks"].set_axon_ntff_profile_hook(hook)
        TRACE = True
        return True
    except Exception as e:  # profiling is best-effort
        print(f"enable_profiling failed: {e}")
        return False


@functools.lru_cache(maxsize=None)
def _stats_prog():
    nc = bacc.Bacc("TRN2")
    xT = nc.declare_dram_parameter("xT", [D, ROWS], f32, isOutput=False)
    memT = nc.declare_dram_parameter("memT", [D, P], f32, isOutput=False)
    # maug[p, h, :] = concat(memory, ones)[h*128 + p, :]
    maug = nc.declare_dram_parameter("maug", [128, 2, D + 1], f32, isOutput=False)
    # raw readout + denominator column; row t*128+p lives at [p, t, :]
    out1r = nc.declare_dram_parameter("out1r", [128, RT, D + 1], f32, isOutput=True)
    vals = nc.declare_dram_parameter("vals", [128, RT * 2], f32, isOutput=True)
    idxs = nc.declare_dram_parameter("idxs", [128, RT * 2], u32, isOutput=True)

    with ExitStack() as ctx:
        tc = ctx.enter_context(tile.TileContext(nc))
        const = ctx.enter_context(tc.tile_pool(name="const", bufs=1))
        sp = ctx.enter_context(tc.tile_pool(name="sp", bufs=3))
        pp = ctx.enter_context(tc.tile_pool(name="pp", bufs=3))
        st = ctx.enter_context(tc.tile_pool(name="st", bufs=1))
        ps = ctx.enter_context(tc.tile_pool(name="ps", bufs=2, space="PSUM"))
        ps2 = ctx.enter_context(tc.tile_pool(name="ps2", bufs=3, space="PSUM"))

        memT_sb = const.tile([D, P], f32)
        nc.sync.dma_start(memT_sb[:], memT[:])
        maug_sb = const.tile([128, 2, D + 1], f32)
        nc.sync.dma_start(maug_sb[:], maug[:])
        xT_sb = const.tile([D, ROWS], f32)
        nc.sync.dma_start(xT_sb[:], xT[:])

        vals_sb = st.tile([128, RT * 2], f32)
        idxs_sb = st.tile([128, RT * 2], u32)
        o_blk = st.tile([128, RT, D + 1], f32)

        for t in range(RT):
            xs = xT_sb[:, t * 128:(t + 1) * 128]

            # scores, row-major: [128 rows, 256 f]
            ps_s = ps.tile([128, P], f32, tag="ps_s")
            nc.tensor.matmul(ps_s[:], xs, memT_sb[:], start=True, stop=True)
            # all PSUM readers stay on ACT so PE's ACT wait covers slot reuse
            s_sb = sp.tile([128, P], f32)
            nc.scalar.copy(s_sb[:], ps_s[:])

            top8 = sp.tile([128, 8], f32)
            nc.vector.max(top8[:], s_sb[:])
            idx8 = sp.tile([128, 8], u32)
            nc.vector.max_index(idx8[:], top8[:], s_sb[:])
            nc.vector.tensor_copy(vals_sb[:, t * 2:t * 2 + 2], top8[:, 0:2])
            nc.vector.tensor_copy(idxs_sb[:, t * 2:t * 2 + 2], idx8[:, 0:2])

            # scores, transposed: [256 f, 128 rows] in one PSUM bank -> exp
            ps_sT = ps2.tile([128, 2, 128], f32, tag="ps_sT")
            for h in range(2):
                nc.tensor.matmul(
                    ps_sT[:, h, :], memT_sb[:, h * 128:(h + 1) * 128], xs,
                    start=True, stop=True,
                )
            pT = pp.tile([128, 2, 128], f32)
            nc.scalar.activation(pT[:], ps_sT[:], AF.Exp)

            # readout + denominator: [128 rows, 129]
            ps_o = ps2.tile([128, D + 1], f32, tag="ps_o")
            nc.tensor.matmul(ps_o[:], pT[:, 0, :], maug_sb[:, 0, :], start=True, stop=False)
            nc.tensor.matmul(ps_o[:], pT[:, 1, :], maug_sb[:, 1, :], start=False, stop=True)
            nc.scalar.copy(o_blk[:, t, :], ps_o[:])

        nc.sync.dma_start(out1r[:], o_blk[:])
        nc.sync.dma_start(vals[:], vals_sb[:])
        nc.sync.dma_start(idxs[:], idxs_sb[:])
    nc.compile()
    return nc


# tiles handled by DVE is_equal; the rest go to ACT via the exact integer
# indicator relu(1 - (x-c)^2) (two activations). DVE ~11.7us/tile,
# ACT ~2x13.9us/tile.
DVE_TILES = 12


@functools.lru_cache(maxsize=None)
def _mask_prog():
    nc = bacc.Bacc("TRN2")
    t1all = nc.declare_dram_parameter("t1all", [1, N], u8, isOutput=False)
    # t1own[p, t, 0] = t1 of row t*128+p of this core's block; [.., 1] = -t1
    t1own = nc.declare_dram_parameter("t1own", [128, RT, 2], f32, isOutput=False)
    mask = nc.declare_dram_parameter("mask", [ROWS, N], u8, isOutput=True)

    with ExitStack() as ctx:
        tc = ctx.enter_context(tile.TileContext(nc))
        const = ctx.enter_context(tc.tile_pool(name="const", bufs=1))
        mpd = ctx.enter_context(tc.tile_pool(name="mpd", bufs=3))
        mpa = ctx.enter_context(tc.tile_pool(name="mpa", bufs=2))
        sqp = ctx.enter_context(tc.tile_pool(name="sqp", bufs=2))
        sqh_p = ctx.enter_context(tc.tile_pool(name="sqh_p", bufs=1))

        t1rep = const.tile([128, N], u8)
        nc.sync.dma_start(t1rep[:], t1all[0:1, :].partition_broadcast(128))
        t1own_sb = const.tile([128, RT, 2], f32)
        nc.sync.dma_start(t1own_sb[:], t1own[:])

        # warmups: let each compute engine observe the input DMA lanes once
        wa = const.tile([128, 8], u8)
        nc.vector.tensor_copy(wa[:], t1rep[:, 0:8])
        wb = const.tile([128, 1], f32)
        nc.vector.tensor_copy(wb[:], t1own_sb[:, 0:1, 0])
        wc = const.tile([128, 8], bf16)
        nc.scalar.activation(wc[:], t1rep[:, 0:8], AF.Square,
                             bias=t1own_sb[:, 0:1, 1])
        wd = const.tile([128, 1], f32)
        nc.scalar.copy(wd[:], t1own_sb[:, 0:1, 0])

        H = N // 2
        for t in range(RT):
            if t < DVE_TILES:
                m_sb = mpd.tile([128, N], u8, tag="md")
                nc.vector.tensor_scalar(
                    m_sb[:], t1rep[:], t1own_sb[:, t:t + 1, 0], None, ALU.is_equal
                )
            elif t == DVE_TILES:
                # split tile: DVE does the left half, ACT the right half
                m_sb = mpd.tile([128, N], u8, tag="md")
                nc.vector.tensor_scalar(
                    m_sb[:, 0:H], t1rep[:, 0:H], t1own_sb[:, t:t + 1, 0],
                    None, ALU.is_equal
                )
                sqh = sqh_p.tile([128, H], bf16, tag="sqh")
                nc.scalar.activation(sqh[:], t1rep[:, H:N], AF.Square,
                                     bias=t1own_sb[:, t:t + 1, 1])
                nc.scalar.activation(m_sb[:, H:N], sqh[:], AF.Relu,
                                     bias=1.0, scale=-1.0)
            else:
                sq = sqp.tile([128, N], bf16, tag="sq")
                nc.scalar.activation(sq[:], t1rep[:], AF.Square,
                                     bias=t1own_sb[:, t:t + 1, 1])
                m_sb = mpa.tile([128, N], u8, tag="ma")
                nc.scalar.activation(m_sb[:], sq[:], AF.Relu,
                                     bias=1.0, scale=-1.0)
            nc.sync.dma_start(mask[t * 128:(t + 1) * 128, :], m_sb[:])
    nc.compile()
    return nc


def _run(nc, in_maps, label):
    res = run_bass_kernel_spmd(nc, in_maps, list(range(NCORES)), trace=TRACE)
    if TRACE:
        LAST[label] = res.exec_time_ns
        LAST_RESULTS[label] = res
    return res.results


def kernel(repres, memory):
    repres = np.ascontiguousarray(np.asarray(repres, dtype=np.float32))
    memory = np.ascontiguousarray(np.asarray(memory, dtype=np.float32))

    memT = np.ascontiguousarray(memory.T)                                  # [128, 256]
    maug = np.concatenate([memory, np.ones((P, 1), np.float32)], axis=1)   # [256, 129]
    maug = np.ascontiguousarray(maug.reshape(2, 128, D + 1).transpose(1, 0, 2))

    in_maps = [
        {
            "xT": np.ascontiguousarray(repres[c * ROWS:(c + 1) * ROWS].T),
            "memT": memT,
            "maug": maug,
        }
        for c in range(NCORES)
    ]
    resA = _run(_stats_prog(), in_maps, "stats")

    out1 = np.empty((N, D), np.float32)
    m12 = np.empty((N, 2), np.float32)
    t12 = np.empty((N, 2), np.int64)
    for c in range(NCORES):
        r = resA[c]
        sl = slice(c * ROWS, (c + 1) * ROWS)
        raw = r["out1r"].transpose(1, 0, 2).reshape(ROWS, D + 1)
        out1[sl] = raw[:, :D] / raw[:, D:D + 1]
        m12[sl] = r["vals"].reshape(128, RT, 2).transpose(1, 0, 2).reshape(ROWS, 2)
        t12[sl] = (
            r["idxs"].reshape(128, RT, 2).transpose(1, 0, 2).reshape(ROWS, 2)
        )

    t1 = t12[:, 0]
    t2 = t12[:, 1]

    # distances from score stats: d^2 = ||x||^2 - 2*s[t] + ||m_t||^2
    r2 = np.einsum("nd,nd->n", repres, repres).astype(np.float32)
    mn2 = np.einsum("pd,pd->p", memory, memory).astype(np.float32)
    d1 = np.sqrt(np.maximum(r2 - 2.0 * m12[:, 0] + mn2[t1], 0.0).astype(np.float32))
    d2 = np.sqrt(np.maximum(r2 - 2.0 * m12[:, 1] + mn2[t2], 0.0).astype(np.float32))
    d1 = (d1 / np.float32(D)).astype(np.float32)
    d2 = (d2 / np.float32(D)).astype(np.float32)

    loss = np.float32(np.mean(d1))
    diff = (d1 - d2 + np.float32(0.001)).astype(np.float32)
    neg = diff < 0
    cnt = int(neg.sum())
    if cnt > 0:
        loss = np.float32(loss + np.float32(diff[neg].sum()) / np.float32(cnt))
    loss = np.float32(loss + np.float32(np.sqrt(np.sum(memory * memory))))

    out = np.concatenate([out1, repres], axis=1)

    t1u8 = np.ascontiguousarray(t1.astype(np.uint8).reshape(1, N))
    own_blocks = [
        t1[c * ROWS:(c + 1) * ROWS].reshape(RT, 128).T.astype(np.float32)
        for c in range(NCORES)
    ]
    in_maps = [
        {
            "t1all": t1u8,
            "t1own": np.ascontiguousarray(
                np.stack([ownT, -ownT], axis=-1).astype(np.float32)
            ),
        }
        for ownT in own_blocks
    ]
    resB = _run(_mask_prog(), in_maps, "mask")

    mask = np.empty((N, N), np.uint8)
    for c in range(NCORES):
        mask[c * ROWS:(c + 1) * ROWS] = resB[c]["mask"]
    return out, loss, mask.view(np.bool_)


# revision 19
# speedup vs baseline: 8.9706x; 1.0002x over previous
"""Trainium2 Bass kernel for nn_MemoryModule (vq_codebook).

reference semantics (N=16384, D=128, P=256):
    s         = repres @ memory.T                      [N, P]
    attention = softmax(s, axis=1)
    output    = attention @ memory                     [N, D]
    t1, t2    = top-2 indices of attention (== top-2 of s; softmax monotone)
    d_i       = ||repres - memory[t_i]|| / D
    loss      = mean(d1) + masked-mean(d1-d2+1e-3 | <0) + ||memory||_F
    out       = concat([output, repres], axis=1)       [N, 2D]
    mask      = t1[:, None] == t1[None, :]             [N, N] bool

Sharding: data-parallel over rows, 2048 rows/core on 8 cores.

Two SPMD launches:
  A (stats):  per 128-row tile: one matmul for scores (row-major) feeding
     max/max_index (top-2 values+indices), two matmuls for transposed scores
     feeding exp() (softmax numerator already in [P, rows] layout so it can be
     the stationary operand of the readout matmul), readout matmul against
     memory augmented with a ones column (gives the softmax denominator for
     free), normalize.  exp() needs no max-subtraction: |scores| <~ 25.
  host: top-1/2 values+indices -> distances via
     d^2 = ||x||^2 - 2*max(s) + ||m_t||^2, scalar loss; gather t1 of all rows.
  B (mask):  replicate t1(all rows) across partitions once, then one
     tensor_scalar is_equal per 128-row tile -> [128, N] uint8, DMA out.
     This is HBM-write-bound: 33.5 MB/core.
"""

import functools
import sys
from contextlib import ExitStack

import numpy as np

sys.path.insert(0, "/opt/trn_rl_repo")

import concourse.bass as bass
import concourse.bacc as bacc
import concourse.mybir as mybir
from concourse import tile
from concourse.bass_utils import run_bass_kernel_spmd

N, D, P = 16384, 128, 256
NCORES = 8
ROWS = N // NCORES      # 2048 rows per core
RT = ROWS // 128        # 16 tiles of 128 rows per core

f32 = mybir.dt.float32
bf16 = mybir.dt.bfloat16
u8 = mybir.dt.uint8
u32 = mybir.dt.uint32
AF = mybir.ActivationFunctionType
ALU = mybir.AluOpType

TRACE = False
LAST = {}
LAST_RESULTS = {}


def enable_profiling():
    """Register the NTFF profile hook (needed for trace=True under axon)."""
    global TRACE
    try:
        import importlib.util
        import types

        if "antenv.axon_hooks" not in sys.modules:
            mod = types.ModuleType("antenv.axon_hooks")
            mod._HOOK = None
            mod.set_axon_ntff_profile_hook = lambda h: setattr(mod, "_HOOK", h)
            mod.get_axon_ntff_profile_hook = lambda: mod._HOOK
            sys.modules["antenv.axon_hooks"] = mod
            import antenv

            antenv.axon_hooks = mod

        spec = importlib.util.spec_from_file_location(
            "trn_boot", "/root/.axon_site/trn_agent_boot/trn_boot.py"
        )
        trn_boot = importlib.util.module_from_spec(spec)
        spec.loader.exec_module(trn_boot)

        hook = trn_boot._ntff_profile_via_ctypes("/opt/axon/libaxon_pjrt.so")
        sys.modules["antenv.axon_hooks"].set_axon_ntff_profile_hook(hook)
        TRACE = True
        return True
    except Exception as e:  # profiling is best-effort
        print(f"enable_profiling failed: {e}")
        return False


@functools.lru_cache(maxsize=None)
def _stats_prog():
    nc = bacc.Bacc("TRN2")
    xT = nc.declare_dram_parameter("xT", [D, ROWS], f32, isOutput=False)
    memT = nc.declare_dram_parameter("memT", [D, P], f32, isOutput=False)
    # maug[p, h, :] = concat(memory, ones)[h*128 + p, :]
    maug = nc.declare_dram_parameter("maug", [128, 2, D + 1], f32, isOutput=False)
    # raw readout + denominator column; row t*128+p lives at [p, t, :]
    out1r = nc.declare_dram_parameter("out1r", [128, RT, D + 1], f32, isOutput=True)
    vals = nc.declare_dram_parameter("vals", [128, RT * 2], f32, isOutput=True)
    idxs = nc.declare_dram_parameter("idxs", [128, RT * 2], u32, isOutput=True)

    with ExitStack() as ctx:
        tc = ctx.enter_context(tile.TileContext(nc))
        const = ctx.enter_context(tc.tile_pool(name="const", bufs=1))
        sp = ctx.enter_context(tc.tile_pool(name="sp", bufs=3))
        pp = ctx.enter_context(tc.tile_pool(name="pp", bufs=3))
        st = ctx.enter_context(tc.tile_pool(name="st", bufs=1))
        ps = ctx.enter_context(tc.tile_pool(name="ps", bufs=2, space="PSUM"))
        ps2 = ctx.enter_context(tc.tile_pool(name="ps2", bufs=3, space="PSUM"))

        memT_sb = const.tile([D, P], f32)
        nc.sync.dma_start(memT_sb[:], memT[:])
        maug_sb = const.tile([128, 2, D + 1], f32)
        nc.sync.dma_start(maug_sb[:], maug[:])
        xT_sb = const.tile([D, ROWS], f32)
        nc.sync.dma_start(xT_sb[:], xT[:])

        vals_sb = st.tile([128, RT * 2], f32)
        idxs_sb = st.tile([128, RT * 2], u32)
        o_blk = st.tile([128, RT, D + 1], f32)

        for t in range(RT):
            xs = xT_sb[:, t * 128:(t + 1) * 128]

            # scores, row-major: [128 rows, 256 f]
            ps_s = ps.tile([128, P], f32, tag="ps_s")
            nc.tensor.matmul(ps_s[:], xs, memT_sb[:], start=True, stop=True)
            # all PSUM readers stay on ACT so PE's ACT wait covers slot reuse
            s_sb = sp.tile([128, P], f32)
            nc.scalar.copy(s_sb[:], ps_s[:])

            top8 = sp.tile([128, 8], f32)
            nc.vector.max(top8[:], s_sb[:])
            idx8 = sp.tile([128, 8], u32)
            nc.vector.max_index(idx8[:], top8[:], s_sb[:])
            nc.vector.tensor_copy(vals_sb[:, t * 2:t * 2 + 2], top8[:, 0:2])
            nc.vector.tensor_copy(idxs_sb[:, t * 2:t * 2 + 2], idx8[:, 0:2])

            # scores, transposed: [256 f, 128 rows] in one PSUM bank -> exp
            ps_sT = ps2.tile([128, 2, 128], f32, tag="ps_sT")
            for h in range(2):
                nc.tensor.matmul(
                    ps_sT[:, h, :], memT_sb[:, h * 128:(h + 1) * 128], xs,
                    start=True, stop=True,
                )
            pT = pp.tile([128, 2, 128], f32)
            nc.scalar.activation(pT[:], ps_sT[:], AF.Exp)

            # readout + denominator: [128 rows, 129]
            ps_o = ps2.tile([128, D + 1], f32, tag="ps_o")
            nc.tensor.matmul(ps_o[:], pT[:, 0, :], maug_sb[:, 0, :], start=True, stop=False)
            nc.tensor.matmul(ps_o[:], pT[:, 1, :], maug_sb[:, 1, :], start=False, stop=True)
            nc.scalar.copy(o_blk[:, t, :], ps_o[:])

        nc.sync.dma_start(out1r[:], o_blk[:])
        nc.sync.dma_start(vals[:], vals_sb[:])
        nc.sync.dma_start(idxs[:], idxs_sb[:])
    nc.compile()
    return nc


# tiles handled by DVE is_equal; the rest go to ACT via the exact integer
# indicator relu(1 - (x-c)^2) (two activations). DVE ~11.7us/tile,
# ACT ~2x13.9us/tile.
DVE_TILES = 12


@functools.lru_cache(maxsize=None)
def _mask_prog():
    nc = bacc.Bacc("TRN2")
    # t1 of all rows, pre-replicated across partitions on the host
    t1all = nc.declare_dram_parameter("t1all", [128, N], u8, isOutput=False)
    # t1own[p, t, 0] = t1 of row t*128+p of this core's block; [.., 1] = -t1
    t1own = nc.declare_dram_parameter("t1own", [128, RT, 2], f32, isOutput=False)
    mask = nc.declare_dram_parameter("mask", [ROWS, N], u8, isOutput=True)

    with ExitStack() as ctx:
        tc = ctx.enter_context(tile.TileContext(nc))
        const = ctx.enter_context(tc.tile_pool(name="const", bufs=1))
        mpd = ctx.enter_context(tc.tile_pool(name="mpd", bufs=3))
        mpa = ctx.enter_context(tc.tile_pool(name="mpa", bufs=2))
        sqp = ctx.enter_context(tc.tile_pool(name="sqp", bufs=2))
        sqh_p = ctx.enter_context(tc.tile_pool(name="sqh_p", bufs=1))

        t1rep = const.tile([128, N], u8)
        nc.sync.dma_start(t1rep[:], t1all[:])
        t1own_sb = const.tile([128, RT, 2], f32)
        nc.sync.dma_start(t1own_sb[:], t1own[:])

        # trigger the ACT spline-table load early (overlaps the t1rep DMA)
        wz = const.tile([128, 8], f32)
        nc.gpsimd.memset(wz[:], 0.0)
        wc = const.tile([128, 8], bf16)
        nc.scalar.activation(wc[:], wz[:], AF.Square)

        H = 3072
        for t in range(RT):
            if t < DVE_TILES:
                m_sb = mpd.tile([128, N], u8, tag="md")
                nc.vector.tensor_scalar(
                    m_sb[:], t1rep[:], t1own_sb[:, t:t + 1, 0], None, ALU.is_equal
                )
            elif t == DVE_TILES:
                # split tile: DVE does the left half, ACT the right half
                m_sb = mpd.tile([128, N], u8, tag="md")
                nc.vector.tensor_scalar(
                    m_sb[:, 0:H], t1rep[:, 0:H], t1own_sb[:, t:t + 1, 0],
                    None, ALU.is_equal
                )
                sqh = sqh_p.tile([128, N - H], bf16, tag="sqh")
                nc.scalar.activation(sqh[:], t1rep[:, H:N], AF.Square,
                                     bias=t1own_sb[:, t:t + 1, 1])
                nc.scalar.activation(m_sb[:, H:N], sqh[:], AF.Relu,
                                     bias=1.0, scale=-1.0)
            else:
                sq = sqp.tile([128, N], bf16, tag="sq")
                nc.scalar.activation(sq[:], t1rep[:], AF.Square,
                                     bias=t1own_sb[:, t:t + 1, 1])
                m_sb = mpa.tile([128, N], u8, tag="ma")
                nc.scalar.activation(m_sb[:], sq[:], AF.Relu,
                                     bias=1.0, scale=-1.0)
            nc.sync.dma_start(mask[t * 128:(t + 1) * 128, :], m_sb[:])
    nc.compile()
    return nc


def _run(nc, in_maps, label):
    res = run_bass_kernel_spmd(nc, in_maps, list(range(NCORES)), trace=TRACE)
    if TRACE:
        LAST[label] = res.exec_time_ns
        LAST_RESULTS[label] = res
    return res.results


def kernel(repres, memory):
    repres = np.ascontiguousarray(np.asarray(repres, dtype=np.float32))
    memory = np.ascontiguousarray(np.asarray(memory, dtype=np.float32))

    memT = np.ascontiguousarray(memory.T)                                  # [128, 256]
    maug = np.concatenate([memory, np.ones((P, 1), np.float32)], axis=1)   # [256, 129]
    maug = np.ascontiguousarray(maug.reshape(2, 128, D + 1).transpose(1, 0, 2))

    in_maps = [
        {
            "xT": np.ascontiguousarray(repres[c * ROWS:(c + 1) * ROWS].T),
            "memT": memT,
            "maug": maug,
        }
        for c in range(NCORES)
    ]
    resA = _run(_stats_prog(), in_maps, "stats")

    out1 = np.empty((N, D), np.float32)
    m12 = np.empty((N, 2), np.float32)
    t12 = np.empty((N, 2), np.int64)
    for c in range(NCORES):
        r = resA[c]
        sl = slice(c * ROWS, (c + 1) * ROWS)
        raw = r["out1r"].transpose(1, 0, 2).reshape(ROWS, D + 1)
        out1[sl] = raw[:, :D] / raw[:, D:D + 1]
        m12[sl] = r["vals"].reshape(128, RT, 2).transpose(1, 0, 2).reshape(ROWS, 2)
        t12[sl] = (
            r["idxs"].reshape(128, RT, 2).transpose(1, 0, 2).reshape(ROWS, 2)
        )

    t1 = t12[:, 0]
    t2 = t12[:, 1]

    # distances from score stats: d^2 = ||x||^2 - 2*s[t] + ||m_t||^2
    r2 = np.einsum("nd,nd->n", repres, repres).astype(np.float32)
    mn2 = np.einsum("pd,pd->p", memory, memory).astype(np.float32)
    d1 = np.sqrt(np.maximum(r2 - 2.0 * m12[:, 0] + mn2[t1], 0.0).astype(np.float32))
    d2 = np.sqrt(np.maximum(r2 - 2.0 * m12[:, 1] + mn2[t2], 0.0).astype(np.float32))
    d1 = (d1 / np.float32(D)).astype(np.float32)
    d2 = (d2 / np.float32(D)).astype(np.float32)

    loss = np.float32(np.mean(d1))
    diff = (d1 - d2 + np.float32(0.001)).astype(np.float32)
    neg = diff < 0
    cnt = int(neg.sum())
    if cnt > 0:
        loss = np.float32(loss + np.float32(diff[neg].sum()) / np.float32(cnt))
    loss = np.float32(loss + np.float32(np.sqrt(np.sum(memory * memory))))

    out = np.concatenate([out1, repres], axis=1)

    t1u8 = t1.astype(np.uint8)
    t1rep_host = np.ascontiguousarray(np.broadcast_to(t1u8[None, :], (128, N)))
    own_blocks = [
        t1[c * ROWS:(c + 1) * ROWS].reshape(RT, 128).T.astype(np.float32)
        for c in range(NCORES)
    ]
    in_maps = [
        {
            "t1all": t1rep_host,
            "t1own": np.ascontiguousarray(
                np.stack([ownT, -ownT], axis=-1).astype(np.float32)
            ),
        }
        for ownT in own_blocks
    ]
    resB = _run(_mask_prog(), in_maps, "mask")

    mask = np.empty((N, N), np.uint8)
    for c in range(NCORES):
        mask[c * ROWS:(c + 1) * ROWS] = resB[c]["mask"]
    return out, loss, mask.view(np.bool_)
